# revision 4
# baseline (speedup 1.0000x reference)
"""Trainium2 Bass kernel for nn_Attention_4243427688485.

Computation (per batch b):
    a   = z_b @ M @ e_b^T            [N, ME]
    A   = softmax(sigmoid(a), dim=N) (softmax over the query axis N)
    eo  = A @ e_b                    [N, D]
Returns (eo, A) stacked over the batch.

Sharding: data-parallel over batch B=8 across the 8 NeuronCores (one batch
per core, M replicated).  No collectives needed.

Per-core implementation notes:
  - All three 1024^3 matmuls run in fp16 (full TensorE rate; fp32 PSUM
    accumulation).  Empirically rel_err ~2.8e-3 vs the f32 reference.
  - sigmoid(x) = 0.5 + 0.5*tanh(x/2) and exp(sigmoid(x)) = exp(0.5*tanh(x/2)
    + 0.5), so the whole softmax-of-sigmoid needs only Tanh and Exp, which
    live in the same ScalarE activation table set (no table thrash).
  - exp's accum_out gives the per-partition row sum (softmax denominator)
    for free.
  - Scores are computed transposed (aT[m, n]) so the softmax reduction is
    along the free axis; A is transposed back on the TensorE for output.
"""

import numpy as np

import concourse.bass as bass
import concourse.mybir as mybir
import concourse.tile as tile
from concourse import bacc
from concourse.bass_utils import run_bass_kernel_spmd
from concourse.masks import make_identity

AF = mybir.ActivationFunctionType
F32 = mybir.dt.float32
FP16 = mybir.dt.float16

P = 128          # partitions
NT = 8           # 1024 / 128 tiles per dim
SZ = 1024        # all matrix dims
NC = 8           # cores / batches


def _build_nc() -> bass.Bass:
    nc = bacc.Bacc()

    z_d = nc.declare_dram_parameter("z", [SZ, SZ], F32, isOutput=False)
    e_d = nc.declare_dram_parameter("e", [SZ, SZ], F32, isOutput=False)
    m_d = nc.declare_dram_parameter("M", [SZ, SZ], F32, isOutput=False)
    eo_d = nc.declare_dram_parameter("eo", [SZ, SZ], F32, isOutput=True)
    a_d = nc.declare_dram_parameter("A", [SZ, SZ], F32, isOutput=True)

    zr = z_d.rearrange("(j p) d -> j p d", p=P)
    er = e_d.rearrange("(j p) d -> j p d", p=P)
    mr = m_d.rearrange("(j p) d -> j p d", p=P)
    eor = eo_d.rearrange("(j p) d -> j p d", p=P)
    ar = a_d.rearrange("(j p) d -> j p d", p=P)

    with tile.TileContext(nc) as tc:
        with (
            tc.tile_pool(name="big", bufs=1) as big,
            tc.tile_pool(name="consts", bufs=1) as consts,
            tc.tile_pool(name="tpool", bufs=3) as tpool,
            tc.tile_pool(name="stage", bufs=4) as stage,
            tc.tile_pool(name="psum_mm", bufs=4, space="PSUM") as pmm,
            tc.tile_pool(name="psum_tr", bufs=2, space="PSUM") as ptr,
        ):
            ident = consts.tile([P, P], FP16)
            make_identity(nc, ident)
            halfb = consts.tile([P, 1], F32)
            nc.any.memset(halfb, 0.5)
            zerob = consts.tile([P, 1], F32)
            nc.any.memset(zerob, 0.0)
            S = consts.tile([P, NT], F32)
            r = consts.tile([P, NT], F32)

            m16 = big.tile([P, NT, SZ], FP16)   # M16[p, jd, e'] = M[jd*128+p, e']
            z16 = big.tile([P, NT, SZ], FP16)   # z16[p, jn, d]  = z[jn*128+p, d]
            e16 = big.tile([P, NT, SZ], FP16)   # e16[p, jm, d]  = e[jm*128+p, d]
            zT = big.tile([P, NT, SZ], FP16)    # zT[p, jd, n]   = z[n, jd*128+p]
            eT = big.tile([P, NT, SZ], FP16)    # eT[p, je, m]   = e[m, je*128+p]
            zMT = big.tile([P, NT, SZ], FP16)   # zMT[p, je, n]  = (z@M)[n, je*128+p]
            u16 = big.tile([P, NT, SZ], FP16)   # u[p, jm, n]    = tanh(a[n, jm*128+p]/2)
            aT16 = big.tile([P, NT, SZ], FP16)  # aT16[p, jm, n] = A[n, jm*128+p]

            # ---- loads (SWDGE cast f32 -> fp16), in first-use order ----
            for j in range(4):
                nc.gpsimd.dma_start(out=z16[:, j, :], in_=zr[j])
            for j in range(NT):
                nc.gpsimd.dma_start(out=m16[:, j, :], in_=mr[j])
            for j in range(4, NT):
                nc.gpsimd.dma_start(out=z16[:, j, :], in_=zr[j])
            for j in range(NT):
                nc.gpsimd.dma_start(out=e16[:, j, :], in_=er[j])

            def transpose_pack(src, dst, jd, h):
                # fills dst[:, jd, h*512:(h+1)*512] from src blocks jn=h*4..h*4+3
                pt = ptr.tile([P, 512], FP16, tag="tr")
                for q in range(4):
                    jn = h * 4 + q
                    nc.tensor.transpose(
                        pt[:, q * P:(q + 1) * P],
                        src[:, jn, jd * P:(jd + 1) * P],
                        ident,
                    )
                nc.vector.tensor_copy(out=dst[:, jd, h * 512:(h + 1) * 512], in_=pt[:])

            # ---- transposes of z (h=0 first so mm1 h=0 can start early) ----
            for jd in range(NT):
                transpose_pack(z16, zT, jd, 0)
            for jd in range(NT):
                transpose_pack(z16, zT, jd, 1)

            # ---- mm1: zMT[e', n] = sum_d M[d, e'] * z[n, d] ----
            for h in range(2):
                for je in range(NT):
                    ps = pmm.tile([P, 512], F32, tag="mm")
                    for jd in range(NT):
                        nc.tensor.matmul(
                            ps[:],
                            m16[:, jd, je * P:(je + 1) * P],
                            zT[:, jd, h * 512:(h + 1) * 512],
                            start=(jd == 0),
                            stop=(jd == NT - 1),
                        )
                    nc.scalar.copy(out=zMT[:, je, h * 512:(h + 1) * 512], in_=ps[:])

            # ---- transposes of e (needed by mm2) ----
            for jd in range(NT):
                for h in range(2):
                    transpose_pack(e16, eT, jd, h)

            # ---- mm2 + fused softmax(sigmoid) per m-tile ----
            for jm in range(NT):
                for h in range(2):
                    ps = pmm.tile([P, 512], F32, tag="mm")
                    for je in range(NT):
                        nc.tensor.matmul(
                            ps[:],
                            eT[:, je, jm * P:(jm + 1) * P],
                            zMT[:, je, h * 512:(h + 1) * 512],
                            start=(je == 0),
                            stop=(je == NT - 1),
                        )
                    # u = tanh(a/2)  (sigmoid(a) = 0.5 + 0.5*u)
                    nc.scalar.activation(
                        u16[:, jm, h * 512:(h + 1) * 512], ps[:], AF.Tanh,
                        bias=zerob[:], scale=0.5,
                    )
                # t = exp(0.5*u + 0.5) = exp(sigmoid(a)); accum -> S
                t = tpool.tile([P, SZ], FP16, tag="t")
                nc.scalar.activation(
                    t[:], u16[:, jm, :], AF.Exp,
                    bias=halfb[:], scale=0.5,
                    accum_out=S[:, jm:jm + 1],
                )
                nc.vector.reciprocal(r[:, jm:jm + 1], S[:, jm:jm + 1])
                nc.vector.tensor_scalar_mul(aT16[:, jm, :], t[:], r[:, jm:jm + 1])

                # once the first half of m-tiles is done, transpose A for output
                if jm == 3:
                    emit_a_out(nc, tc, ptr, stage, ar, aT16, ident, 0)

            # ---- mm3: eo[n, d] = sum_m A[n, m] * e[m, d]; interleave A output ----
            for jn in range(NT):
                for h2 in range(2):
                    ps = pmm.tile([P, 512], F32, tag="mm")
                    for jm in range(NT):
                        nc.tensor.matmul(
                            ps[:],
                            aT16[:, jm, jn * P:(jn + 1) * P],
                            e16[:, jm, h2 * 512:(h2 + 1) * 512],
                            start=(jm == 0),
                            stop=(jm == NT - 1),
                        )
                    st = stage.tile([P, 512], F32, tag="eost")
                    nc.scalar.copy(out=st[:], in_=ps[:])
                    nc.sync.dma_start(out=eor[jn, :, h2 * 512:(h2 + 1) * 512], in_=st[:])
                if jn == 1:
                    emit_a_out(nc, tc, ptr, stage, ar, aT16, ident, 1)

    nc.compile()
    return nc


def emit_a_out(nc, tc, ptr, stage, ar, aT16, ident, g):
    """Transpose A columns m = g*512 .. g*512+512 back to [n, m] and DMA out."""
    for jn in range(NT):
        pa = ptr.tile([P, 512], FP16, tag="tr")
        for q in range(4):
            jm = g * 4 + q
            nc.tensor.transpose(
                pa[:, q * P:(q + 1) * P],
                aT16[:, jm, jn * P:(jn + 1) * P],
                ident,
            )
        st = stage.tile([P, 512], F32, tag="ast")
        nc.scalar.copy(out=st[:], in_=pa[:])
        nc.sync.dma_start(out=ar[jn, :, g * 512:(g + 1) * 512], in_=st[:])


_NC_CACHE = None


def _get_nc():
    global _NC_CACHE
    if _NC_CACHE is None:
        _NC_CACHE = _build_nc()
    return _NC_CACHE


def kernel(z: np.ndarray, e: np.ndarray, M: np.ndarray):
    z = np.ascontiguousarray(np.asarray(z, dtype=np.float32))
    e = np.ascontiguousarray(np.asarray(e, dtype=np.float32))
    M = np.ascontiguousarray(np.asarray(M, dtype=np.float32))
    assert z.shape == (NC, SZ, SZ) and e.shape == (NC, SZ, SZ) and M.shape == (SZ, SZ)

    nc = _get_nc()
    in_maps = [{"z": z[i], "e": e[i], "M": M} for i in range(NC)]
    res = run_bass_kernel_spmd(nc, in_maps, core_ids=list(range(NC))).results
    eo = np.stack([res[i]["eo"] for i in range(NC)])
    A = np.stack([res[i]["A"] for i in range(NC)])
    return eo, A


# revision 10
# speedup vs baseline: 192.1242x; 192.1242x over previous
"""Trainium2 Bass kernel for nn_Attention_4243427688485.

Computation (per batch b):
    a   = z_b @ M @ e_b^T            [N, ME]
    A   = softmax(sigmoid(a), dim=N) (softmax over the query axis N)
    eo  = A @ e_b                    [N, D]
Returns (eo, A) stacked over the batch.

Sharding: data-parallel over batch B=8 across the 8 NeuronCores (one batch
per core, M replicated).  No collectives needed.

Per-core implementation notes:
  - All three 1024^3 matmuls run in fp16 (full TensorE rate; fp32 PSUM
    accumulation).  Empirically rel_err ~2.8e-3 vs the f32 reference.
  - sigmoid(x) = 0.5 + 0.5*tanh(x/2) and exp(sigmoid(x)) = exp(0.5*tanh(x/2)
    + 0.5), so the whole softmax-of-sigmoid needs only Tanh and Exp, which
    live in the same ScalarE activation table set (no table thrash).
  - exp's accum_out gives the per-partition row sum (softmax denominator)
    for free.
  - Scores are computed transposed (aT[m, n]) so the softmax reduction is
    along the free axis; A is transposed back on the TensorE for output.
"""

import numpy as np

import concourse.bass as bass
import concourse.mybir as mybir
import concourse.tile as tile
from concourse import bacc
from concourse.bass_utils import run_bass_kernel_spmd
from concourse.masks import make_identity

AF = mybir.ActivationFunctionType
F32 = mybir.dt.float32
FP16 = mybir.dt.float16

P = 128          # partitions
NT = 8           # 1024 / 128 tiles per dim
SZ = 1024        # all matrix dims
NC = 8           # cores / batches


def _build_nc(unroll: int = 1, tiny_io: bool = False) -> bass.Bass:
    """Build the per-core program.

    unroll/tiny_io are for timing only: tiny_io replaces the big external
    tensors with internal DRAM scratch (plus a [1,1] dummy in/out) so the
    per-call host marshalling cost vanishes, and unroll repeats the body K
    times inside one NEFF so the HW time dominates dispatch overhead.
    """
    nc = bacc.Bacc()

    if tiny_io:
        nc.declare_dram_parameter("tin", [1, 1], F32, isOutput=False)
        dout = nc.declare_dram_parameter("tout", [1, 1], F32, isOutput=True)
        z_d = nc.dram_tensor("zi", [SZ, SZ], F32)
        e_d = nc.dram_tensor("ei", [SZ, SZ], F32)
        m_d = nc.dram_tensor("Mi", [SZ, SZ], F32)
        eo_d = nc.dram_tensor("eoi", [SZ, SZ], F32)
        a_d = nc.dram_tensor("Ai", [SZ, SZ], F32)
    else:
        z_d = nc.declare_dram_parameter("z", [SZ, SZ], F32, isOutput=False)
        e_d = nc.declare_dram_parameter("e", [SZ, SZ], F32, isOutput=False)
        m_d = nc.declare_dram_parameter("M", [SZ, SZ], F32, isOutput=False)
        eo_d = nc.declare_dram_parameter("eo", [SZ, SZ], F32, isOutput=True)
        a_d = nc.declare_dram_parameter("A", [SZ, SZ], F32, isOutput=True)

    zr = z_d.rearrange("(j p) d -> j p d", p=P)
    er = e_d.rearrange("(j p) d -> j p d", p=P)
    mr = m_d.rearrange("(j p) d -> j p d", p=P)
    eor = eo_d.rearrange("(j p) d -> j p d", p=P)
    ar = a_d.rearrange("(j p) d -> j p d", p=P)

    with tile.TileContext(nc) as tc:
        with (
            tc.tile_pool(name="big", bufs=1) as big,
            tc.tile_pool(name="consts", bufs=1) as consts,
            tc.tile_pool(name="tpool", bufs=3) as tpool,
            tc.tile_pool(name="stage", bufs=4) as stage,
            tc.tile_pool(name="psum_mm", bufs=4, space="PSUM") as pmm,
            tc.tile_pool(name="psum_tr", bufs=2, space="PSUM") as ptr,
        ):
            ident = consts.tile([P, P], FP16)
            make_identity(nc, ident)
            halfb = consts.tile([P, 1], F32)
            nc.any.memset(halfb, 0.5)
            zerob = consts.tile([P, 1], F32)
            nc.any.memset(zerob, 0.0)
            S = consts.tile([P, NT], F32)
            r = consts.tile([P, NT], F32)

            m16 = big.tile([P, NT, SZ], FP16)   # M16[p, jd, e'] = M[jd*128+p, e']
            z16 = big.tile([P, NT, SZ], FP16)   # z16[p, jn, d]  = z[jn*128+p, d]
            e16 = big.tile([P, NT, SZ], FP16)   # e16[p, jm, d]  = e[jm*128+p, d]
            zT = big.tile([P, NT, SZ], FP16)    # zT[p, jd, n]   = z[n, jd*128+p]
            eT = big.tile([P, NT, SZ], FP16)    # eT[p, je, m]   = e[m, je*128+p]
            zMT = big.tile([P, NT, SZ], FP16)   # zMT[p, je, n]  = (z@M)[n, je*128+p]
            u16 = big.tile([P, NT, SZ], FP16)   # u[p, jm, n]    = tanh(a[n, jm*128+p]/2)
            aT16 = big.tile([P, NT, SZ], FP16)  # aT16[p, jm, n] = A[n, jm*128+p]

            for _ in range(unroll):
                _emit_body(
                    nc, ptr, pmm, tpool, stage,
                    zr, er, mr, eor, ar,
                    m16, z16, e16, zT, eT, zMT, u16, aT16,
                    ident, halfb, zerob, S, r,
                )

            if tiny_io:
                dstage = consts.tile([1, 1], F32)
                nc.any.memset(dstage, 1.0)
                nc.sync.dma_start(out=dout[:], in_=dstage[:])

    nc.compile()
    return nc


def _emit_body(nc, ptr, pmm, tpool, stage, zr, er, mr, eor, ar,
               m16, z16, e16, zT, eT, zMT, u16, aT16,
               ident, halfb, zerob, S, r):
            # ---- loads (SWDGE cast f32 -> fp16), in first-use order ----
            for j in range(4):
                nc.gpsimd.dma_start(out=z16[:, j, :], in_=zr[j])
            for j in range(NT):
                nc.gpsimd.dma_start(out=m16[:, j, :], in_=mr[j])
            for j in range(4, NT):
                nc.gpsimd.dma_start(out=z16[:, j, :], in_=zr[j])
            for j in range(NT):
                nc.gpsimd.dma_start(out=e16[:, j, :], in_=er[j])

            def transpose_pack(src, dst, jd, h):
                # fills dst[:, jd, h*512:(h+1)*512] from src blocks jn=h*4..h*4+3
                pt = ptr.tile([P, 512], FP16, tag="tr")
                for q in range(4):
                    jn = h * 4 + q
                    nc.tensor.transpose(
                        pt[:, q * P:(q + 1) * P],
                        src[:, jn, jd * P:(jd + 1) * P],
                        ident,
                    )
                nc.vector.tensor_copy(out=dst[:, jd, h * 512:(h + 1) * 512], in_=pt[:])

            # ---- transposes of z (h=0 first so mm1 h=0 can start early) ----
            for jd in range(NT):
                transpose_pack(z16, zT, jd, 0)
            for jd in range(NT):
                transpose_pack(z16, zT, jd, 1)

            # ---- mm1: zMT[e', n] = sum_d M[d, e'] * z[n, d] ----
            for h in range(2):
                for je in range(NT):
                    ps = pmm.tile([P, 512], F32, tag="mm")
                    for jd in range(NT):
                        nc.tensor.matmul(
                            ps[:],
                            m16[:, jd, je * P:(je + 1) * P],
                            zT[:, jd, h * 512:(h + 1) * 512],
                            start=(jd == 0),
                            stop=(jd == NT - 1),
                        )
                    nc.scalar.copy(out=zMT[:, je, h * 512:(h + 1) * 512], in_=ps[:])

            # ---- transposes of e (needed by mm2) ----
            for jd in range(NT):
                for h in range(2):
                    transpose_pack(e16, eT, jd, h)

            # ---- mm2 + fused softmax(sigmoid) per m-tile ----
            for jm in range(NT):
                for h in range(2):
                    ps = pmm.tile([P, 512], F32, tag="mm")
                    for je in range(NT):
                        nc.tensor.matmul(
                            ps[:],
                            eT[:, je, jm * P:(jm + 1) * P],
                            zMT[:, je, h * 512:(h + 1) * 512],
                            start=(je == 0),
                            stop=(je == NT - 1),
                        )
                    # u = tanh(a/2)  (sigmoid(a) = 0.5 + 0.5*u)
                    nc.scalar.activation(
                        u16[:, jm, h * 512:(h + 1) * 512], ps[:], AF.Tanh,
                        bias=zerob[:], scale=0.5,
                    )
                # t = exp(0.5*u + 0.5) = exp(sigmoid(a)); accum -> S
                t = tpool.tile([P, SZ], FP16, tag="t")
                nc.scalar.activation(
                    t[:], u16[:, jm, :], AF.Exp,
                    bias=halfb[:], scale=0.5,
                    accum_out=S[:, jm:jm + 1],
                )
                nc.vector.reciprocal(r[:, jm:jm + 1], S[:, jm:jm + 1])
                nc.vector.tensor_scalar_mul(aT16[:, jm, :], t[:], r[:, jm:jm + 1])

                # once the first half of m-tiles is done, transpose A for output
                if jm == 3:
                    emit_a_out(nc, ptr, stage, ar, aT16, ident, 0)

            # ---- mm3: eo[n, d] = sum_m A[n, m] * e[m, d]; interleave A output ----
            for jn in range(NT):
                for h2 in range(2):
                    ps = pmm.tile([P, 512], F32, tag="mm")
                    for jm in range(NT):
                        nc.tensor.matmul(
                            ps[:],
                            aT16[:, jm, jn * P:(jn + 1) * P],
                            e16[:, jm, h2 * 512:(h2 + 1) * 512],
                            start=(jm == 0),
                            stop=(jm == NT - 1),
                        )
                    st = stage.tile([P, 512], F32, tag="eost")
                    nc.scalar.copy(out=st[:], in_=ps[:])
                    nc.sync.dma_start(out=eor[jn, :, h2 * 512:(h2 + 1) * 512], in_=st[:])
                if jn == 1:
                    emit_a_out(nc, ptr, stage, ar, aT16, ident, 1)


def emit_a_out(nc, ptr, stage, ar, aT16, ident, g):
    """Transpose A columns m = g*512 .. g*512+512 back to [n, m] and DMA out."""
    for jn in range(NT):
        pa = ptr.tile([P, 512], FP16, tag="tr")
        for q in range(4):
            jm = g * 4 + q
            nc.tensor.transpose(
                pa[:, q * P:(q + 1) * P],
                aT16[:, jm, jn * P:(jn + 1) * P],
                ident,
            )
        st = stage.tile([P, 512], F32, tag="ast")
        nc.scalar.copy(out=st[:], in_=pa[:])
        nc.sync.dma_start(out=ar[jn, :, g * 512:(g + 1) * 512], in_=st[:])


_NC_CACHE = None


def _get_nc():
    global _NC_CACHE
    if _NC_CACHE is None:
        _NC_CACHE = _build_nc()
    return _NC_CACHE


def kernel(z: np.ndarray, e: np.ndarray, M: np.ndarray):
    z = np.ascontiguousarray(np.asarray(z, dtype=np.float32))
    e = np.ascontiguousarray(np.asarray(e, dtype=np.float32))
    M = np.ascontiguousarray(np.asarray(M, dtype=np.float32))
    assert z.shape == (NC, SZ, SZ) and e.shape == (NC, SZ, SZ) and M.shape == (SZ, SZ)

    nc = _get_nc()
    in_maps = [{"z": z[i], "e": e[i], "M": M} for i in range(NC)]
    res = run_bass_kernel_spmd(nc, in_maps, core_ids=list(range(NC))).results
    eo = np.stack([res[i]["eo"] for i in range(NC)])
    A = np.stack([res[i]["A"] for i in range(NC)])
    return eo, A
